# revision 17
# baseline (speedup 1.0000x reference)
"""Trainium2 Bass kernel for nn_Graph_Enhance_model (GNN message passing).

Self-contained: hardcodes shapes B=4,F=32,H=8,O=16,D=2048, 8 cores.

Phase A (edge waves): data-parallel over the 128 (b,f) frames, 16/core.
  Step-1 wave exploits UM0's structure: its msg_n half is broadcast over h,
  so the wave is a K=1024 matmul plus a rank-64 PSUM update built from
  Q = msg_n @ Wcat[1024:] and the step-0 softmax weights.
Phases B/C (GRUs): tensor-parallel over the 2048 hidden units, 256/core;
  each core computes ALL 128 frames for its unit slice. M_sum, All_human
  and s1 are exchanged with small HBM AllGathers (8-core mesh, ~5-15us).
"""

import os
import sys

for _p in ("/opt/trn_rl_repo", "/opt/pypackages"):
    if _p not in sys.path and os.path.isdir(_p):
        sys.path.append(_p)

import numpy as np
import ml_dtypes

import concourse.bass as bass
import concourse.bacc as bacc
import concourse.tile as tile
import concourse.mybir as mybir
from concourse import bass_utils
from concourse.masks import make_identity

BF16 = mybir.dt.bfloat16
F8 = mybir.dt.float8e4
F32 = mybir.dt.float32
AF = mybir.ActivationFunctionType
ALU = mybir.AluOpType
AX = mybir.AxisListType

NB = ml_dtypes.bfloat16
N8 = ml_dtypes.float8_e4m3

B, F, H, O, D = 4, 32, 8, 16, 2048
NFRAMES = B * F          # 128
NCORES = 8
FPC = NFRAMES // NCORES  # 16 frames per core
ROWS = H * O             # 128 rows per frame
KC = D // 128            # 16 K-chunks
NQ = FPC // 4            # 4 quads of 4 frames
UPC = D // NCORES        # 256 units per core (TP slice)
GPC = 3 * UPC            # 768 gate columns per core
NR = NFRAMES * H         # 1024 human rows globally
RCN = NR // 128          # 8 row chunks

_CACHE = {}
RG = [list(range(NCORES))]


def _build_nc():
    nc = bacc.Bacc("TRN2", target_bir_lowering=False, debug=False, num_devices=NCORES)

    dt_in = {}

    def din(name, shape, dt):
        dt_in[name] = nc.dram_tensor(name, shape, dt, kind="ExternalInput")
        return dt_in[name]

    # per-core phase A (partition-major layouts: contiguous per-partition DMA)
    e0t = din("e0t", [NQ, 128, KC, 512], F8)
    ot = din("ot", [128, KC, FPC * O], BF16)
    # replicated phase A consts (wcat pre-scaled x32 in fp8)
    wcat = din("wcat", [128, KC, D], F8)
    bl1td = din("bl1t", [128, 8], F32)
    bet0d = din("bet0", [128, 8], F32)
    bet1d = din("bet1", [128, 8], F32)
    wnt = din("wnt", [4, 128, KC, 256], BF16)
    wnb = din("wnb", [1, D // 2], BF16)
    wl2 = din("wl2", [128, 8, 1], BF16)
    scatd = din("scat2", [128, 2, 512], BF16)
    # phase B (TP slices + replicated transposed inputs)
    pmatd = din("pmat", [128, FPC], BF16)
    htfd = din("ht_full", [RCN, 128, KC, 128], F8)
    hrmd = din("h_rm_s", [128, RCN, UPC], F32)
    whid = din("whi_s", [128, KC, GPC], F8)
    whhd = din("whh_s", [128, KC, GPC], F8)
    bhid = din("bhi_s", [1, GPC], BF16)
    bhhd = din("bhh_s", [1, GPC], BF16)
    # phase C
    wsid = din("wsi_s", [128, KC, GPC], BF16)
    wshd = din("wsh_s", [128, KC, GPC], BF16)
    bsid = din("bsi_s", [1, GPC], BF16)
    bshd = din("bsh_s", [1, GPC], BF16)
    sc4td = din("sc4t", [128, KC, NFRAMES], BF16)
    sftd = din("sft", [128, KC, NFRAMES], BF16)
    sc4sd = din("sc4_s", [NFRAMES, UPC], F32)
    sfsd = din("sf_s", [NFRAMES, UPC], F32)
    outp = nc.dram_tensor("outp", [NFRAMES, UPC], F32, kind="ExternalOutput")

    from contextlib import ExitStack

    with tile.TileContext(nc) as tc, ExitStack() as ctx:
        glob = ctx.enter_context(tc.tile_pool(name="glob", bufs=1))
        dram = ctx.enter_context(tc.tile_pool(name="dram", bufs=1, space="DRAM"))

        # DRAM bounce buffers for collectives (M_sum gathered per quad so the
        # AllGathers overlap the remaining quads' compute); partition-major so
        # both the bounce write and the msT reads are contiguous per partition
        msum_cin = [dram.tile([128, KC, 32], F8, name=f"msum_cin{q}") for q in range(NQ)]
        msum_cout = [dram.tile([NCORES * 128, KC, 32], F8, addr_space="Shared",
                               name=f"msum_cout{q}") for q in range(NQ)]
        ah_cin = dram.tile([UPC, NFRAMES], BF16)
        ah_cout = dram.tile([D, NFRAMES], BF16, addr_space="Shared")
        s1_cin = dram.tile([UPC, NFRAMES], BF16)
        s1_cout = dram.tile([D, NFRAMES], BF16, addr_space="Shared")

        ones_b = glob.tile([1, 512], BF16)
        nc.vector.memset(ones_b, 1.0)
        ident128 = glob.tile([128, 128], BF16)
        make_identity(nc, ident128)

        wl2_sb = glob.tile([128, 8, 1], BF16)
        bl1t_sb = glob.tile([128, 8], F32)
        bet0_sb = glob.tile([128, 8], F32)
        bet1_sb = glob.tile([128, 8], F32)
        ones_s0 = glob.tile([1, 128], BF16)
        nc.vector.memset(ones_s0, 0.5)          # w/2 broadcast (step-0 combines)
        ones_s1 = glob.tile([1, 128], BF16)
        nc.vector.memset(ones_s1, 1.0 / 512.0)  # w/512 broadcast (step-1 combines)
        s2048 = glob.tile([128, 1], F32)
        nc.vector.memset(s2048, 1.0 / 32768.0)  # descale for phase-B gi psums (incl /O)
        scat_sb = glob.tile([128, 2, 512], BF16)
        pmat_sb = glob.tile([128, FPC], BF16)

        msgn_sb = glob.tile([128, 8, FPC * O], F8)      # msg_n^T [1024, 256] (for Q)
        msgn_b = glob.tile([128, 8, FPC * O], BF16)     # bf16 copy (vector mn path)
        msum_f = glob.tile([128, KC, 128], F32)         # M_sum^T local (sum over o)

        # phase B weights: resident whole kernel; loaded on sync queue after
        # the phase-0/A critical loads (emission order below)
        whi_sb = glob.tile([128, KC, GPC], F8)
        whh_sb = glob.tile([128, KC, GPC], F8)
        bhi_sb = glob.tile([1, GPC], BF16)
        bhh_sb = glob.tile([1, GPC], BF16)

        # phase C weights: prefetch on gpsimd during phase A, ahead of
        # the per-quad AllGather triggers on that queue. A dummy copy that
        # reads the last phase-0 output gates the queue so these transfers
        # don't compete with the phase-0/A critical loads at t=0.
        pcw = ctx.enter_context(tc.tile_pool(name="pcw", bufs=1))
        gate_dram = dram.tile([1, 16], BF16)
        wsh_sb = pcw.tile([128, KC, GPC], BF16)
        bsi_sb = pcw.tile([1, GPC], BF16)
        bsh_sb = pcw.tile([1, GPC], BF16)
        sc4t_sb = pcw.tile([128, KC, NFRAMES], BF16)
        sft_sb = pcw.tile([128, KC, NFRAMES], BF16)
        sc4s_sb = pcw.tile([NFRAMES, UPC], F32)
        sfs_sb = pcw.tile([NFRAMES, UPC], F32)

        def load_phase_c_weights():
            # gate: waits for the last phase-0 msgn write, keeping these
            # transfers off the t=0 critical DMA window
            nc.gpsimd.dma_start(out=gate_dram, in_=msgn_b[0:1, 7, 0:16])
            nc.gpsimd.dma_start(out=wsh_sb, in_=wshd.ap())
            nc.gpsimd.dma_start(out=bsi_sb, in_=bsid.ap())
            nc.gpsimd.dma_start(out=bsh_sb, in_=bshd.ap())
            nc.gpsimd.dma_start(out=sc4t_sb, in_=sc4td.ap())
            nc.gpsimd.dma_start(out=sft_sb, in_=sftd.ap())
            nc.gpsimd.dma_start(out=sc4s_sb, in_=sc4sd.ap())
            nc.gpsimd.dma_start(out=sfs_sb, in_=sfsd.ap())

        with tc.tile_pool(name="paq", bufs=1) as paq:
            q_sb = paq.tile([128, 2, D], F8)            # Q for quad-pairs (x32)

            with (
                tc.tile_pool(name="pwcat", bufs=1) as pwcat,
                tc.tile_pool(name="pa", bufs=1) as pa,
                tc.tile_pool(name="pa1", bufs=1) as pa1,
            ):
                wcat_sb = pwcat.tile([128, KC, D], F8)

                # ---------------- Phase 0: msg_n^T = Wn @ O^T + bn ----------------
                with nc.named_scope("ph0"):
                    with (
                        tc.tile_pool(name="p0", bufs=1) as p0,
                        tc.tile_pool(name="p0ps", bufs=4, space="PSUM") as p0ps,
                    ):
                        # sync-queue DMA order = phase-0 feeds, wcat (hi half
                        # first for Q), first quad's edges, then B weights
                        # ph0's critical feeds go FIRST on the fast sync queue
                        # (~190GB/s); wcat follows and lands while ph0's MMs
                        # drain. Scalar queue carries wn1-3 + small consts.
                        wnb_sb = p0.tile([1, D // 2], BF16)
                        nc.sync.dma_start(out=wnb_sb, in_=wnb.ap())
                        ot_sb = p0.tile([128, KC, FPC * O], BF16)
                        nc.sync.dma_start(out=ot_sb, in_=ot.ap())
                        wn_t = {}

                        def wn_load(qr, eng=None):
                            wn_t[qr] = p0.tile([128, KC, 256], BF16, tag="wn", bufs=4,
                                               name=f"wn{qr}")
                            (eng or nc.scalar).dma_start(out=wn_t[qr], in_=wnt.ap()[qr])

                        wn_load(0, nc.sync)
                        # wn1-3 spread over scalar+gpsimd so ph0's later
                        # quarters don't stall behind one slow queue
                        wn_load(1, nc.scalar)
                        wn_load(2, nc.gpsimd)
                        wn_load(3, nc.gpsimd)
                        nc.sync.dma_start(out=wcat_sb[:, 8:16, :], in_=wcat.ap()[:, 8:16, :])
                        nc.sync.dma_start(out=wcat_sb[:, 0:8, :], in_=wcat.ap()[:, 0:8, :])
                        xq0 = pa.tile([128, KC, 512], F8, tag="xq")
                        nc.sync.dma_start(out=xq0, in_=e0t.ap()[0])
                        nc.scalar.dma_start(out=wl2_sb, in_=wl2.ap())
                        nc.scalar.dma_start(out=bl1t_sb, in_=bl1td.ap())
                        nc.scalar.dma_start(out=bet0_sb, in_=bet0d.ap())
                        nc.scalar.dma_start(out=bet1_sb, in_=bet1d.ap())
                        nc.scalar.dma_start(out=scat_sb, in_=scatd.ap())
                        nc.scalar.dma_start(out=pmat_sb, in_=pmatd.ap())
                        nc.scalar.dma_start(out=whi_sb, in_=whid.ap())
                        nc.scalar.dma_start(out=whh_sb, in_=whhd.ap())
                        nc.scalar.dma_start(out=bhi_sb, in_=bhid.ap())
                        nc.scalar.dma_start(out=bhh_sb, in_=bhhd.ap())

                        for quar in range(4):
                            wn_sb = wn_t[quar]
                            for mt2 in range(2):
                                mt = quar * 2 + mt2
                                pm = p0ps.tile([128, FPC * O], F32, tag="pm")
                                for kc in range(KC):
                                    nc.tensor.matmul(pm, lhsT=wn_sb[:, kc, mt2 * 128:(mt2 + 1) * 128],
                                                     rhs=ot_sb[:, kc, :], start=(kc == 0), stop=False)
                                nc.tensor.matmul(pm, lhsT=wnb_sb[0:1, mt * 128:(mt + 1) * 128],
                                                 rhs=ones_b[0:1, 0:FPC * O], start=False, stop=True)
                                nc.scalar.copy(msgn_sb[:, mt, :], pm)
                                nc.scalar.copy(msgn_b[:, mt, :], pm)

                load_phase_c_weights()

                # ---------------- Q = msg_n @ Wcat[1024:, :]  (for step-1 rank update) ----
                with nc.named_scope("phQ"):
                    with tc.tile_pool(name="pqps", bufs=2, space="PSUM") as pqps:
                        for qq in range(2):
                            for ms in range(4):
                                pqp = pqps.tile([128, 512], F32, tag="pqp")
                                for j2 in range(4):
                                    nc.tensor.matmul(pqp,
                                                     lhsT=msgn_sb[:, 2 * j2:2 * j2 + 2, qq * 128:(qq + 1) * 128],
                                                     rhs=wcat_sb[:, 8 + 2 * j2:10 + 2 * j2, ms * 512:(ms + 1) * 512],
                                                     start=(j2 == 0), stop=(j2 == 3),
                                                     perf_mode=mybir.MatmulPerfMode.DoubleRow)
                                nc.scalar.copy(q_sb[:, qq, ms * 512:(ms + 1) * 512], pqp)

                # ---------------- Phase A: 2 propagation steps over edges ----------------
                with tc.tile_pool(name="paps", bufs=4, space="PSUM") as paps, \
                     tc.tile_pool(name="papss", bufs=1, space="PSUM") as papss:
                    for q in range(NQ):
                        if q == 0:
                            xq = xq0
                        else:
                            xq = pa.tile([128, KC, 512], F8, tag="xq")
                            nc.sync.dma_start(out=xq, in_=e0t.ap()[q])
                        um1t = pa1.tile([128, 8, 512], F8, tag="um1t")
                        wscat = pa1.tile([128, 512], F8, tag="wscat")
                        for step in range(2):
                            # last quad's step 1 broadcasts w right after the
                            # softmax (small PE stall) so the msum tail +
                            # AllGather fire as early as possible
                            early_w = (step == 1 and q == NQ - 1)
                            with nc.named_scope(f"q{q}s{step}"):
                                def chain(pt, mt):
                                    if step == 0:
                                        for k2 in range(8):
                                            nc.tensor.matmul(pt,
                                                             lhsT=wcat_sb[:, 2 * k2:2 * k2 + 2, mt * 128:(mt + 1) * 128],
                                                             rhs=xq[:, 2 * k2:2 * k2 + 2, :],
                                                             start=(k2 == 0), stop=(k2 == 7),
                                                             perf_mode=mybir.MatmulPerfMode.DoubleRow)
                                    else:
                                        for k2 in range(4):
                                            nc.tensor.matmul(pt,
                                                             lhsT=wcat_sb[:, 2 * k2:2 * k2 + 2, mt * 128:(mt + 1) * 128],
                                                             rhs=um1t[:, 2 * k2:2 * k2 + 2, :],
                                                             start=(k2 == 0), stop=False,
                                                             perf_mode=mybir.MatmulPerfMode.DoubleRow)
                                        nc.tensor.matmul(pt, lhsT=q_sb[:, q // 2, mt * 128:(mt + 1) * 128],
                                                         rhs=wscat, start=False, stop=True)

                                # --- a-wave: relu(X @ Wl1^T + bl1), transposed ---
                                psc = 1.0 / 32.0 if step == 0 else 1.0 / 512.0
                                relu_sb = pa1.tile([128, 8, 512], BF16, tag="relu")
                                for mt in range(8, 16):
                                    pw_a = paps.tile([128, 512], F32, tag="wave")
                                    chain(pw_a, mt)
                                    nc.scalar.activation(relu_sb[:, mt - 8, :], pw_a, AF.Relu,
                                                         bias=bl1t_sb[:, mt - 8:mt - 7], scale=psc)
                                # --- logits + softmax over o (groups of 16) ---
                                pl = papss.tile([1, 512], F32, tag="pl")
                                for kc2 in range(8):
                                    nc.tensor.matmul(pl, lhsT=wl2_sb[:, kc2, :],
                                                     rhs=relu_sb[:, kc2, :], start=(kc2 == 0), stop=(kc2 == 7))
                                pl3 = pl.rearrange("o (g i) -> o g i", i=16)
                                mx = pa1.tile([1, 32], F32, tag="mx")
                                nc.vector.reduce_max(mx, pl3, axis=AX.X)
                                sub = pa1.tile([1, 512], F32, tag="sub")
                                nc.vector.tensor_tensor(sub.rearrange("o (g i) -> o g i", i=16), pl3,
                                                        mx.broadcast_to((1, 32, 16)), op=ALU.subtract)
                                nc.scalar.activation(sub, sub, AF.Exp)
                                ex3 = sub.rearrange("o (g i) -> o g i", i=16)
                                sm = pa1.tile([1, 32], F32, tag="sm")
                                nc.vector.reduce_sum(sm, ex3, axis=AX.X)
                                rs = pa1.tile([1, 32], F32, tag="rs")
                                nc.vector.reciprocal(rs, sm)
                                w_sb = pa1.tile([1, 512], BF16, tag="w")
                                nc.vector.tensor_tensor(w_sb.rearrange("o (g i) -> o g i", i=16), ex3,
                                                        rs.broadcast_to((1, 32, 16)), op=ALU.mult)
                                # --- msg_e wave; w-broadcast MM emitted after 2 groups ---
                                e_ps = []
                                wb_sb = pa1.tile([128, 512], F32, tag="wb")
                                wbs_sb = pa1.tile([128, 512], F32, tag="wbs")

                                def bcast_w():
                                    pw_b = papss.tile([128, 512], F32, tag="pw")
                                    nc.tensor.matmul(pw_b, lhsT=ones_b[0:1, 0:128], rhs=w_sb,
                                                     start=True, stop=True)
                                    nc.scalar.copy(wb_sb, pw_b)
                                    ssrc = ones_s0 if step == 0 else ones_s1
                                    pw_s = papss.tile([128, 512], F32, tag="pws")
                                    nc.tensor.matmul(pw_s, lhsT=ssrc, rhs=w_sb,
                                                     start=True, stop=True)
                                    nc.scalar.copy(wbs_sb, pw_s)

                                def msgn_half():
                                    # msg_n half of M_sum: w1-weighted msg_n summed over o.
                                    # On the last quad the multiplies run on gpsimd so they
                                    # overlap the e-wave combines on the vector queue and the
                                    # final msum AllGather fires earlier.
                                    wb4 = wb_sb.rearrange("p (f h o) -> p f h o", f=4, h=8)
                                    meng = nc.gpsimd if early_w else nc.vector
                                    for j in range(8):
                                        base = msgn_b[:, j, q * 64:(q + 1) * 64]
                                        mn_bc = bass.AP(tensor=base.tensor, offset=base.offset,
                                                        ap=[list(base.ap[0]), [16, 4], [0, 8], [1, 16]])
                                        tmp = pa1.tile([128, 512], F32, tag=f"um2g{j % 2}" if early_w else "um2")
                                        meng.tensor_tensor(
                                            tmp.rearrange("p (f h o) -> p f h o", f=4, h=8),
                                            mn_bc, wb4, op=ALU.mult)
                                        nc.vector.reduce_sum(msum_f[:, 8 + j, q * 32:(q + 1) * 32],
                                                             tmp.rearrange("p (f h o) -> p f h o", f=4, h=8),
                                                             axis=AX.X)

                                if early_w:
                                    bcast_w()
                                    msgn_half()

                                def combine(mt, pe):
                                    if step == 0:
                                        # (32*wave + 32*be) * (w/2) = 16*UM0 -> fp8
                                        nc.vector.scalar_tensor_tensor(
                                            out=um1t[:, mt, :], in0=pe, scalar=bet0_sb[:, mt:mt + 1],
                                            in1=wbs_sb, op0=ALU.add, op1=ALU.mult)
                                    else:
                                        # (512*wave + 512*be) * (w/512) = UM1 exact
                                        tmp = pa1.tile([128, 512], F32, tag="um2")
                                        nc.vector.scalar_tensor_tensor(
                                            out=tmp, in0=pe, scalar=bet1_sb[:, mt:mt + 1],
                                            in1=wbs_sb, op0=ALU.add, op1=ALU.mult)
                                        nc.vector.reduce_sum(msum_f[:, mt, q * 32:(q + 1) * 32],
                                                             tmp.rearrange("p (f h o) -> p f h o", f=4, h=8),
                                                             axis=AX.X)

                                for mt in range(8):
                                    pe = paps.tile([128, 512], F32, tag="wave")
                                    chain(pe, mt)
                                    e_ps.append(pe)
                                    if mt == 1 and not early_w:
                                        # broadcast w along partitions via K=1 matmul (PE waits
                                        # here on softmax, hidden under the first 2 MM groups)
                                        bcast_w()
                                    if mt >= 1:
                                        for cmt in ([0, 1] if mt == 1 else [mt]):
                                            combine(cmt, e_ps[cmt])
                                if step == 0:
                                    # rank-update rhs for step 1: scatter w0 over (f,o) rows
                                    nc.vector.tensor_tensor(wscat, scat_sb[:, q % 2, :], wb_sb, op=ALU.mult)
                                elif not early_w:
                                    msgn_half()
                        # per-quad M_sum bf16 convert + bounce write + AllGather,
                        # overlapped with the remaining quads' compute
                        with nc.named_scope(f"msum_ag{q}"):
                            msb = pa1.tile([128, KC, 32], F8, tag="msb")
                            for kc in range(KC):
                                nc.scalar.activation(msb[:, kc, :], msum_f[:, kc, q * 32:(q + 1) * 32],
                                                     AF.Copy, scale=64.0)
                            nc.gpsimd.dma_start(out=msum_cin[q], in_=msb)
                            nc.gpsimd.collective_compute(
                                "AllGather", ALU.bypass, replica_groups=RG,
                                ins=[msum_cin[q].opt()], outs=[msum_cout[q].opt()])

        # ---------------- Phase B: human GRU, TP over units ----------------
        # phase C's h-side gate chains are emitted right after phBh so they
        # fill the PE idle window while the last msum AG lands; results are
        # parked in SBUF so no PSUM banks stay held through phase B.
        with tc.tile_pool(name="pcgh", bufs=1) as pcgh, \
             tc.tile_pool(name="pcw2", bufs=1) as pcw2:
            # wsi lands in wcat's freed space right after phase A; the sync
            # queue is compute-free so the transfer starts as soon as the
            # space frees, well before phCx1 needs it
            wsi_sb = pcw2.tile([128, KC, GPC], BF16)
            nc.sync.dma_start(out=wsi_sb, in_=wsid.ap())

            def s_gh_part(ht_sb, sfx, pool):
                with nc.named_scope("phCh" + sfx):
                    p_rz = pool.tile([128, 512], F32, tag="ghp", name="ghp" + sfx)
                    for kc in range(KC):
                        nc.tensor.matmul(p_rz, lhsT=ht_sb[:, kc, :], rhs=wsh_sb[:, kc, 0:512],
                                         start=(kc == 0), stop=False)
                    nc.tensor.matmul(p_rz, lhsT=ones_b[0:1, 0:128], rhs=bsh_sb[0:1, 0:512],
                                     start=False, stop=True)
                    p_hn = pool.tile([128, 256], F32, tag="ghq", name="ghq" + sfx)
                    for kc in range(KC):
                        nc.tensor.matmul(p_hn, lhsT=ht_sb[:, kc, :], rhs=wsh_sb[:, kc, 512:768],
                                         start=(kc == 0), stop=False)
                    nc.tensor.matmul(p_hn, lhsT=ones_b[0:1, 0:128], rhs=bsh_sb[0:1, 512:768],
                                     start=False, stop=True)
                    grz_c = pcgh.tile([128, 512], F32, tag="cgrz" + sfx, name="cgrz" + sfx)
                    nc.scalar.copy(grz_c, p_rz)
                    ghn_c = pcgh.tile([128, 256], F32, tag="cghn" + sfx, name="cghn" + sfx)
                    nc.scalar.copy(ghn_c, p_hn)
                    return grz_c, ghn_c

            with (
                tc.tile_pool(name="pbw", bufs=1) as pbw,
                tc.tile_pool(name="pb", bufs=2) as pb,
                tc.tile_pool(name="pb1", bufs=2) as pb1,
            ):
                # H^T row chunks as 8 per-rc tiles (sync queue), each a fully
                # contiguous partition-major DMA
                ht_t = []
                for rc in range(RCN):
                    t = pbw.tile([128, KC, 128], F8, name=f"ht{rc}")
                    nc.sync.dma_start(out=t, in_=htfd.ap()[rc])
                    ht_t.append(t)
                hrm_sb = pbw.tile([128, RCN, UPC], F32)
                nc.sync.dma_start(out=hrm_sb, in_=hrmd.ap())
                # h-dependent gate halves, computed while the msum AGs finish
                ghrz_sb = pbw.tile([128, RCN, 512], BF16)
                ghn_sb = pbw.tile([128, RCN, 256], BF16)

                def ht_chunk(rc):
                    return ht_t[rc], slice(0, 128)

                with nc.named_scope("phBh"):
                    # pure-h gate halves for all row chunks: this PE work
                    # needs no msum, so it overlaps the msum AllGathers
                    with tc.tile_pool(name="pbhps", bufs=2, space="PSUM") as pbhps:
                        for rc in range(RCN):
                            htc, rsl = ht_chunk(rc)
                            p_hrz = pbhps.tile([128, 512], F32, tag="prz")
                            for k2 in range(8):
                                nc.tensor.matmul(p_hrz, lhsT=htc[:, 2 * k2:2 * k2 + 2, rsl],
                                                 rhs=whh_sb[:, 2 * k2:2 * k2 + 2, 0:512],
                                                 start=(k2 == 0), stop=False,
                                                 perf_mode=mybir.MatmulPerfMode.DoubleRow)
                            nc.tensor.matmul(p_hrz, lhsT=ones_b[0:1, 0:128], rhs=bhh_sb[0:1, 0:512],
                                             start=False, stop=True)
                            p_hhn = pbhps.tile([128, 256], F32, tag="phn")
                            for k2 in range(8):
                                nc.tensor.matmul(p_hhn, lhsT=htc[:, 2 * k2:2 * k2 + 2, rsl],
                                                 rhs=whh_sb[:, 2 * k2:2 * k2 + 2, 512:768],
                                                 start=(k2 == 0), stop=False,
                                                 perf_mode=mybir.MatmulPerfMode.DoubleRow)
                            nc.tensor.matmul(p_hhn, lhsT=ones_b[0:1, 0:128], rhs=bhh_sb[0:1, 512:768],
                                             start=False, stop=True)
                            # psums hold 32*gh (+bias staged x32): store true gh
                            nc.scalar.activation(ghrz_sb[:, rc, :], p_hrz, AF.Copy, scale=1.0 / 32.0)
                            nc.scalar.activation(ghn_sb[:, rc, :], p_hhn, AF.Copy, scale=1.0 / 32.0)

                # phase C h-side chains fill the PE window while AG3 lands
                with tc.tile_pool(name="pcghp", bufs=2, space="PSUM") as pcghp:
                    gh_rz1, gh_hn1 = s_gh_part(sc4t_sb, "1", pcghp)
                    gh_rz2, gh_hn2 = s_gh_part(sft_sb, "2", pcghp)

                with tc.tile_pool(name="pbps", bufs=2, space="PSUM") as pbps:
                    pah = pbps.tile([128, 2, 128], F32, tag="pah", bufs=1)

                    with nc.named_scope("phB"):
                        for rc in range(RCN):
                            msT = pb.tile([128, KC, 128], F8, tag="msT")
                            for q in range(NQ):
                                nc.sync.dma_start(
                                    out=msT[:, :, q * 32:(q + 1) * 32],
                                    in_=msum_cout[q][rc * 128:(rc + 1) * 128])
                            p_girz = pbps.tile([128, 512], F32, tag="pgirz")
                            for k2 in range(8):
                                nc.tensor.matmul(p_girz, lhsT=msT[:, 2 * k2:2 * k2 + 2, :],
                                                 rhs=whi_sb[:, 2 * k2:2 * k2 + 2, 0:512],
                                                 start=(k2 == 0), stop=False,
                                                 perf_mode=mybir.MatmulPerfMode.DoubleRow)
                            nc.tensor.matmul(p_girz, lhsT=ones_b[0:1, 0:128],
                                             rhs=bhi_sb[0:1, 0:512], start=False, stop=True)
                            p_gin = pbps.tile([128, 256], F32, tag="pgin")
                            for k2 in range(8):
                                nc.tensor.matmul(p_gin, lhsT=msT[:, 2 * k2:2 * k2 + 2, :],
                                                 rhs=whi_sb[:, 2 * k2:2 * k2 + 2, 512:768],
                                                 start=(k2 == 0), stop=False,
                                                 perf_mode=mybir.MatmulPerfMode.DoubleRow)
                            nc.tensor.matmul(p_gin, lhsT=ones_b[0:1, 0:128],
                                             rhs=bhi_sb[0:1, 512:768], start=False, stop=True)
                            # elementwise GRU combine -> humans (bf16);
                            # gi psums hold 32768*gi (+bias staged x32768)
                            grz = pb1.tile([128, 512], F32, tag="grz")
                            nc.vector.scalar_tensor_tensor(
                                out=grz, in0=p_girz, scalar=s2048,
                                in1=ghrz_sb[:, rc, :], op0=ALU.mult, op1=ALU.add)
                            rz = pb1.tile([128, 512], F32, tag="rz")
                            nc.scalar.activation(rz, grz, AF.Sigmoid)
                            t1 = pb1.tile([128, 256], F32, tag="t1")
                            nc.vector.tensor_tensor(t1, rz[:, 0:256], ghn_sb[:, rc, :], op=ALU.mult)
                            t2 = pb1.tile([128, 256], F32, tag="t2")
                            nc.vector.scalar_tensor_tensor(
                                out=t2, in0=p_gin, scalar=s2048,
                                in1=t1, op0=ALU.mult, op1=ALU.add)
                            n_sb = pb1.tile([128, 256], F32, tag="n")
                            nc.scalar.activation(n_sb, t2, AF.Tanh)
                            t3 = pb1.tile([128, 256], F32, tag="t3")
                            nc.vector.tensor_tensor(t3, hrm_sb[:, rc, :], n_sb, op=ALU.subtract)
                            t4 = pb1.tile([128, 256], F32, tag="t4")
                            nc.vector.tensor_tensor(t4, rz[:, 256:512], t3, op=ALU.mult)
                            hum_bt = pb1.tile([128, 256], BF16, tag="hum")
                            nc.vector.tensor_tensor(hum_bt, n_sb, t4, op=ALU.add)
                            # All_human^T contribution: mean over h via pmat
                            for u2 in range(2):
                                nc.tensor.matmul(pah[:, u2, rc * FPC:(rc + 1) * FPC],
                                                 lhsT=hum_bt[:, u2 * 128:(u2 + 1) * 128],
                                                 rhs=pmat_sb, start=True, stop=True)

                        ahT = pb1.tile([128, 2, 128], BF16, tag="ahT")
                        nc.scalar.copy(ahT[:, 0, :], pah[:, 0, :])
                        nc.scalar.copy(ahT[:, 1, :], pah[:, 1, :])

                with nc.named_scope("ah_ag"):
                    nc.sync.dma_start(out=ah_cin.rearrange("(c p) n -> p c n", p=128), in_=ahT)
                    nc.gpsimd.collective_compute(
                        "AllGather", ALU.bypass, replica_groups=RG,
                        ins=[ah_cin.opt()], outs=[ah_cout.opt()])

            # ---------------- Phase C: two S-node GRUs, TP over units ----------------
            with (
                tc.tile_pool(name="pc1", bufs=1) as pc1,
                tc.tile_pool(name="pcsm", bufs=1) as pcsm,
                tc.tile_pool(name="pcps", bufs=1, space="PSUM") as pcps,
                tc.tile_pool(name="pctps", bufs=1, space="PSUM") as pctps,
            ):
                def s_gi_part(xt_sb, sfx):
                    with nc.named_scope("phCx" + sfx):
                        p_rz = pcps.tile([128, 512], F32, tag="sgz" + sfx, name="sgz" + sfx)
                        for kc in range(KC):
                            nc.tensor.matmul(p_rz, lhsT=xt_sb[:, kc, :], rhs=wsi_sb[:, kc, 0:512],
                                             start=(kc == 0), stop=False)
                        nc.tensor.matmul(p_rz, lhsT=ones_b[0:1, 0:128], rhs=bsi_sb[0:1, 0:512],
                                         start=False, stop=True)
                        p_in = pcps.tile([128, 256], F32, tag="sin" + sfx, name="sin" + sfx)
                        for kc in range(KC):
                            nc.tensor.matmul(p_in, lhsT=xt_sb[:, kc, :], rhs=wsi_sb[:, kc, 512:768],
                                             start=(kc == 0), stop=False)
                        nc.tensor.matmul(p_in, lhsT=ones_b[0:1, 0:128], rhs=bsi_sb[0:1, 512:768],
                                         start=False, stop=True)
                        return p_rz, p_in

                def s_gru_elem(p_giz, gh_rz, p_in, gh_hn, h_sb, out_sb):
                    grs = pcsm.tile([128, 512], F32, tag="grs")
                    nc.vector.tensor_tensor(grs, p_giz, gh_rz, op=ALU.add)
                    rz = pcsm.tile([128, 512], F32, tag="crz")
                    nc.scalar.activation(rz, grs, AF.Sigmoid)
                    u1 = pcsm.tile([128, 256], F32, tag="u1")
                    nc.vector.tensor_tensor(u1, rz[:, 0:256], gh_hn, op=ALU.mult)
                    u2 = pcsm.tile([128, 256], F32, tag="u2")
                    nc.vector.tensor_tensor(u2, u1, p_in, op=ALU.add)
                    n1 = pcsm.tile([128, 256], F32, tag="n1")
                    nc.scalar.activation(n1, u2, AF.Tanh)
                    u3 = pcsm.tile([128, 256], F32, tag="u3")
                    nc.vector.tensor_tensor(u3, h_sb, n1, op=ALU.subtract)
                    u4 = pcsm.tile([128, 256], F32, tag="u4")
                    nc.vector.tensor_tensor(u4, rz[:, 256:512], u3, op=ALU.mult)
                    nc.vector.tensor_tensor(out_sb, n1, u4, op=ALU.add)

                ah_all = pc1.tile([128, KC, 128], BF16)
                nc.sync.dma_start(out=ah_all, in_=ah_cout.rearrange("(kc p) n -> p kc n", p=128))
                p_giz1, p_in1 = s_gi_part(ah_all, "1")
                s1_b = pc1.tile([NFRAMES, UPC], BF16)
                s_gru_elem(p_giz1, gh_rz1, p_in1, gh_hn1, sc4s_sb, s1_b)
                # transpose s1 slice -> [units, frames], gather to full s1^T
                s1T = pc1.tile([128, 2, 128], BF16)
                for u2 in range(2):
                    ptp = pctps.tile([128, 128], BF16, tag="tp")
                    nc.tensor.transpose(ptp, s1_b[:, u2 * 128:(u2 + 1) * 128], ident128)
                    nc.scalar.copy(s1T[:, u2, :], ptp)
                with nc.named_scope("s1_ag"):
                    nc.sync.dma_start(out=s1_cin.rearrange("(c p) n -> p c n", p=128), in_=s1T)
                    nc.gpsimd.collective_compute(
                        "AllGather", ALU.bypass, replica_groups=RG,
                        ins=[s1_cin.opt()], outs=[s1_cout.opt()])
                s1t_all = pc1.tile([128, KC, 128], BF16)
                nc.sync.dma_start(out=s1t_all, in_=s1_cout.rearrange("(kc p) n -> p kc n", p=128))
                p_giz2, p_in2 = s_gi_part(s1t_all, "2")
                out_sb = pc1.tile([NFRAMES, UPC], F32)
                s_gru_elem(p_giz2, gh_rz2, p_in2, gh_hn2, sfs_sb, out_sb)
                nc.sync.dma_start(out=outp.ap(), in_=out_sb)

    nc.compile()
    return nc


def _pm(a):
    """[KC'*128, N] -> partition-major [128, KC', N] contiguous."""
    rows, n = a.shape
    kc = rows // 128
    return np.ascontiguousarray(a.reshape(kc, 128, n).transpose(1, 0, 2))


def _prep_in_maps(inputs):
    E = np.ascontiguousarray(inputs["H_O_edges"].reshape(NFRAMES, ROWS, D))
    On = inputs["O_nodes"].reshape(NFRAMES, O, D)
    Hn = inputs["H_nodes"].reshape(NFRAMES, H, D)
    Sc4 = inputs["S_node_C4"].reshape(NFRAMES, D)
    Sf = np.ascontiguousarray(inputs["final_S_node"].transpose(0, 2, 1)).reshape(NFRAMES, D)

    whi_t = np.ascontiguousarray(inputs["gh_wih"].T)
    whh_t = np.ascontiguousarray(inputs["gh_whh"].T)
    wsi_t = np.ascontiguousarray(inputs["gs_wih"].T)
    wsh_t = np.ascontiguousarray(inputs["gs_whh"].T)

    def slice_gates(Wt, c):
        return np.ascontiguousarray(np.concatenate(
            [Wt[:, g * D + c * UPC:g * D + (c + 1) * UPC] for g in range(3)], axis=1))

    def slice_bias(b, c):
        return np.concatenate(
            [b[g * D + c * UPC:g * D + (c + 1) * UPC] for g in range(3)])[None, :]

    # scat2[:, par, :]: maps quad-column (f,h,o) to Q-pair row (f + 4*par, o)
    # value 16 so wscat = scat*w = 16*w matches the fp8 scale plan
    scat = np.zeros((128, 2, 512), np.float32)
    for par in range(2):
        for f in range(4):
            for h in range(H):
                for o in range(O):
                    scat[(f + 4 * par) * O + o, par, f * 128 + h * O + o] = 16.0

    wnt_full = np.ascontiguousarray(inputs["Wn"].T).astype(NB)  # [D, 1024]
    wnt_q = np.stack([_pm(wnt_full[:, qr * 256:(qr + 1) * 256]) for qr in range(4)])
    htf = np.clip(np.ascontiguousarray(Hn.reshape(NR, D).T), -240, 240).astype(N8)
    htf_rc = np.stack([_pm(htf[:, rc * 128:(rc + 1) * 128]) for rc in range(RCN)])

    shared = {
        "wcat": _pm(np.clip(np.ascontiguousarray(
            np.concatenate([inputs["We"], inputs["Wl1"]], axis=0).T) * 32.0,
            -240, 240).astype(N8)),
        "bl1t": np.ascontiguousarray(inputs["bl1"].reshape(8, 128).T).astype(np.float32),
        "bet0": np.ascontiguousarray(32.0 * inputs["be"].reshape(8, 128).T).astype(np.float32),
        "bet1": np.ascontiguousarray(512.0 * inputs["be"].reshape(8, 128).T).astype(np.float32),
        "pmat": np.ascontiguousarray(np.kron(np.eye(FPC), np.ones((H, 1))) / H).astype(NB),
        "wnt": wnt_q,
        "wnb": inputs["bn"][None, :].astype(NB),
        "wl2": np.ascontiguousarray(inputs["Wl2"][0].reshape(8, 128).T)[:, :, None].astype(NB),
        "scat2": scat.astype(NB),
        "ht_full": htf_rc,
        "sc4t": _pm(np.ascontiguousarray(Sc4.T).astype(NB)),
        "sft": _pm(np.ascontiguousarray(Sf.T).astype(NB)),
    }

    in_maps = []
    for c in range(NCORES):
        fr = slice(c * FPC, (c + 1) * FPC)
        us = slice(c * UPC, (c + 1) * UPC)
        Ec = E[fr]  # [16, 128, 2048]
        e0t = np.clip(np.ascontiguousarray(
            Ec.reshape(NQ, 4, ROWS, D).transpose(0, 3, 1, 2).reshape(NQ, D, 512)), -240, 240).astype(N8)
        e0t = np.ascontiguousarray(
            e0t.reshape(NQ, KC, 128, 512).transpose(0, 2, 1, 3))  # [NQ,128,KC,512]
        ot = _pm(np.ascontiguousarray(On[fr].reshape(FPC * O, D).T).astype(NB))
        hrm = Hn.reshape(NR, D)[:, us].astype(np.float32)  # [NR, UPC]
        hrm = np.ascontiguousarray(hrm.reshape(RCN, 128, UPC).transpose(1, 0, 2))
        m = dict(shared)
        m.update({
            "e0t": e0t,
            "ot": ot,
            "h_rm_s": hrm,
            "whi_s": _pm(np.clip(slice_gates(whi_t, c) * 32.0, -240, 240).astype(N8)),
            "whh_s": _pm(np.clip(slice_gates(whh_t, c) * 32.0, -240, 240).astype(N8)),
            "bhi_s": (slice_bias(inputs["gh_bih"], c) * 32768.0).astype(NB),
            "bhh_s": (slice_bias(inputs["gh_bhh"], c) * 32.0).astype(NB),
            "wsi_s": _pm(slice_gates(wsi_t, c).astype(NB)),
            "wsh_s": _pm(slice_gates(wsh_t, c).astype(NB)),
            "bsi_s": slice_bias(inputs["gs_bih"], c).astype(NB),
            "bsh_s": slice_bias(inputs["gs_bhh"], c).astype(NB),
            "sc4_s": np.ascontiguousarray(Sc4[:, us]).astype(np.float32),
            "sf_s": np.ascontiguousarray(Sf[:, us]).astype(np.float32),
        })
        in_maps.append(m)
    return in_maps


LAST_RESULT = None


def kernel(**inputs):
    global LAST_RESULT
    if "nc" not in _CACHE:
        _CACHE["nc"] = _build_nc()
    nc = _CACHE["nc"]
    in_maps = _prep_in_maps(inputs)
    trace = os.environ.get("KERNEL_TRACE", "0") == "1"
    res = bass_utils.run_bass_kernel_spmd(
        nc, in_maps, core_ids=list(range(NCORES)), trace=trace)
    LAST_RESULT = res
    out = np.concatenate([res.results[c]["outp"] for c in range(NCORES)], axis=1)
    return np.ascontiguousarray(out.reshape(B, F, D)).astype(np.float32)


if __name__ == "__main__":
    np.random.seed(0)
    ins = {
        "S_node_C4": np.random.randn(B, F, D).astype(np.float32),
        "final_S_node": np.random.randn(B, D, F).astype(np.float32),
        "H_nodes": np.random.randn(B, F, H, D).astype(np.float32),
        "O_nodes": np.random.randn(B, F, O, D).astype(np.float32),
        "H_O_edges": np.random.randn(B, F, H, O, D).astype(np.float32),
        "Wn": np.random.randn(D // 2, D).astype(np.float32) * 0.02,
        "bn": np.random.randn(D // 2).astype(np.float32) * 0.02,
        "We": np.random.randn(D // 2, D).astype(np.float32) * 0.02,
        "be": np.random.randn(D // 2).astype(np.float32) * 0.02,
        "Wl1": np.random.randn(D // 2, D).astype(np.float32) * 0.02,
        "bl1": np.random.randn(D // 2).astype(np.float32) * 0.02,
        "Wl2": np.random.randn(1, D // 2).astype(np.float32) * 0.02,
        "bl2": np.random.randn(1).astype(np.float32) * 0.02,
        "gh_wih": np.random.randn(3 * D, D).astype(np.float32) * 0.02,
        "gh_whh": np.random.randn(3 * D, D).astype(np.float32) * 0.02,
        "gh_bih": np.random.randn(3 * D).astype(np.float32) * 0.02,
        "gh_bhh": np.random.randn(3 * D).astype(np.float32) * 0.02,
        "gs_wih": np.random.randn(3 * D, D).astype(np.float32) * 0.02,
        "gs_whh": np.random.randn(3 * D, D).astype(np.float32) * 0.02,
        "gs_bih": np.random.randn(3 * D).astype(np.float32) * 0.02,
        "gs_bhh": np.random.randn(3 * D).astype(np.float32) * 0.02,
    }
    out = kernel(**ins)
    print("kernel ran, out shape", out.shape, out.dtype, float(np.abs(out).mean()))

